# revision 16
# baseline (speedup 1.0000x reference)
"""GuidedAttention Trainium2 kernel — batch-parallel over 8 NeuronCores.

Per core (one batch element, SQ=SK=D=1024, H=16, DH=64):
  q = hs @ Wq.T + bq ; k = ctx @ Wk.T + bk ; v = ctx @ Wv.T + bv
  attn1 = softmax(q k^T / 32) ; gctx = attn1 @ v
  new_q = LN(relu(gctx @ Wobs.T + bobs)) ; new_k = LN(relu(k @ Wmat.T + bmat))
  out = MHA(new_q, new_k, v)  (16 heads of 64)

v3 design (over the v2 all-SBUF bf16 kernel):
  * LN block transposes are ordinary matmuls against a f16 identity
    (stationary = the normalized fp16 block, FWL-eligible) instead of
    is_transpose PE ops — ~2.5x cheaper on PE; affine applied on 4
    batched blocks per PSUM bank.
  * ctx resident in SBUF; Wk streamed exactly once (v2 re-streamed it
    4x); kT emitted as two N=512 column halves.
  * MHA runs as two query-halves D0/D1.  D0 only depends on the first
    LN half, so the tile scheduler overlaps D0's exp stream (the ACT
    bottleneck) with phase B1's PE-bound work.
  * v_aug carries its softmax-sum ones column FIRST, so the per-head
    denominators land on PSUM partition 0 where reciprocal_approx_fast
    can read them directly — no DRAM bounces in the MHA tail.
Output is produced transposed [D, SQ]; the host transposes it back.
"""

import numpy as np

B, SQ, SK, D, H = 8, 1024, 1024, 1024, 16
DH = D // H
LN_EPS = 1e-5
N_CORES = 8

_CACHE = {}


def _build():
    import concourse.mybir as mybir
    import concourse.tile as tile
    from concourse import bacc
    from concourse.masks import make_identity

    f32 = mybir.dt.float32
    f32r = mybir.dt.float32r
    bf16 = mybir.dt.bfloat16
    f16 = mybir.dt.float16
    AF = mybir.ActivationFunctionType
    ALU = mybir.AluOpType

    nc = bacc.Bacc(None, target_bir_lowering=False)

    d_hsT = nc.dram_tensor("hsT", [D, SQ], bf16, kind="ExternalInput")
    d_ctxT = nc.dram_tensor("ctxT", [D, SK], bf16, kind="ExternalInput")
    d_w = {
        n: nc.dram_tensor(n, [D, D], bf16, kind="ExternalInput")
        for n in ("WqT", "WkT", "WvT", "WobsT", "WmatT")
    }
    d_b = {
        n: nc.dram_tensor(n, [D], f32, kind="ExternalInput")
        for n in ("bq", "bk", "bv", "bobs", "bmat", "ln_g", "ln_b")
    }
    d_outT = nc.dram_tensor("outT", [D, SQ], f32, kind="ExternalOutput")
    d_rs = nc.dram_tensor("rs_scratch", [SQ], f32, kind="Internal")
    d_rinv = nc.dram_tensor("rinv_scratch", [H, SQ], f32, kind="Internal")

    def r8(ap):  # [(c p), x] -> [p, c, x]
        return ap.rearrange("(c p) x -> p c x", p=128)

    def vec2d(name):  # [D] -> [1, D] AP
        return d_b[name][:].rearrange("(a d) -> a d", a=1)

    with tile.TileContext(nc) as tc:
        with tc.tile_pool(name="persist", bufs=1) as pp:
            # ---- constants / persistent tiles ----
            v_aug = pp.tile([128, 8, H, 128], bf16, tag="vaug")
            nqT = pp.tile([128, 8, SQ], bf16, tag="nqT")
            nkT = pp.tile([128, 8, SK], bf16, tag="nkT")
            ident = pp.tile([128, 128], f32, tag="ident")
            make_identity(nc, ident[:])
            ident_h = pp.tile([128, 128], f16, tag="ident_h")
            nc.vector.tensor_copy(ident_h[:], ident[:])
            ones_row = pp.tile([1, 128], f32r, tag="ones_row")
            d_ones = nc.inline_tensor(np.ones((1, 128), np.float32), name="ones_const")
            nc.sync.dma_start(ones_row[:], d_ones[:].bitcast(f32r))
            ones_col_bf = pp.tile([128, 1], bf16, tag="ones_col")
            nc.vector.memset(ones_col_bf[:], 1.0)
            bq_sc = pp.tile([128, 8], f32, tag="bq_sc")
            bk_sc = pp.tile([128, 8], f32, tag="bk_sc")
            nc.sync.dma_start(bq_sc[:], d_b["bq"][:].rearrange("(c p) -> p c", p=128))
            nc.sync.dma_start(bk_sc[:], d_b["bk"][:].rearrange("(c p) -> p c", p=128))
            g_sc = pp.tile([128, 8], f32, tag="g_sc")
            b_sc = pp.tile([128, 8], f32, tag="b_sc")
            nc.sync.dma_start(g_sc[:], d_b["ln_g"][:].rearrange("(c p) -> p c", p=128))
            nc.sync.dma_start(b_sc[:], d_b["ln_b"][:].rearrange("(c p) -> p c", p=128))
            bobs_row = pp.tile([1, D], f32r, tag="bobs_row")
            nc.sync.dma_start(bobs_row[:], vec2d("bobs").bitcast(f32r))
            # guide softmax column sums, [s_p, s_c] layout + derived eps'
            Sc = pp.tile([128, 8], f32, tag="Sc")
            eps_q = pp.tile([128, 8], f32, tag="eps_q")
            S_row = pp.tile([1, SQ], f32r, tag="S_row")

            def ln_batch_apply(tmp, n_str, var_all, mean_all, eps_col, tpool,
                               mm_pool, dst, dst_off):
                """tmp [128, n_str, 1024] fp16 relu'd rows (s on partitions),
                per-stripe stats APs [128, n_str].  Normalize in place,
                transpose 128x128 blocks via identity-matmul, then
                per-partition affine (g, b) into dst[:, ot, dst_off...]."""
                lnv = tpool.tile([128, n_str], f32, tag="lnv")
                nc.vector.tensor_tensor(lnv[:], var_all, eps_col, ALU.add)
                std = tpool.tile([128, n_str], f32, tag="std")
                nc.scalar.activation(std[:], lnv[:], AF.Sqrt)
                rstd = tpool.tile([128, n_str], f32, tag="rstd")
                nc.vector.reciprocal(rstd[:], std[:])
                negmr = tpool.tile([128, n_str], f32, tag="negmr")
                nc.vector.tensor_mul(negmr[:], mean_all, rstd[:])
                nc.vector.tensor_scalar_mul(negmr[:], negmr[:], -1.0)
                for i in range(n_str):
                    nc.vector.tensor_scalar(
                        tmp[:, i, :], tmp[:, i, :],
                        rstd[:, i:i + 1], negmr[:, i:i + 1],
                        ALU.mult, ALU.add,
                    )
                # transpose: out[d, s] = sum_s' tmp[s', d] * I[s', s]
                for ot in range(8):
                    pst = mm_pool.tile([128, 512], f32, tag="mm")
                    for i in range(n_str):
                        nc.tensor.matmul(
                            pst[:, i * 128:(i + 1) * 128],
                            tmp[:, i, ot * 128:(ot + 1) * 128],
                            ident_h[:], start=True, stop=True,
                        )
                    nc.vector.tensor_scalar(
                        dst[:, ot, dst_off:dst_off + n_str * 128],
                        pst[:], g_sc[:, ot:ot + 1], b_sc[:, ot:ot + 1],
                        ALU.mult, ALU.add,
                    )

            # ============ shared pools across A+B ============
            pab_cm = tc.tile_pool(name="pab", bufs=1)
            pab = pab_cm.__enter__()
            kT = pab.tile([128, 8, SK], bf16, tag="kT")
            eps_k = pab.tile([128, 8], f32, tag="eps_k")
            nc.vector.memset(eps_k[:], LN_EPS)
            phs_cm = tc.tile_pool(name="p_hs", bufs=2)
            phs = phs_cm.__enter__()
            pwq_cm = tc.tile_pool(name="p_wq", bufs=3)
            pwq = pwq_cm.__enter__()
            ptmp_cm = tc.tile_pool(name="p_tmp", bufs=2)
            ptmp = ptmp_cm.__enter__()
            pmm_cm = tc.tile_pool(name="ps_mm", bufs=3, space="PSUM")
            pap = pmm_cm.__enter__()

            def ln_group(tmp, mv, eps_col, dst, dst_off):
                ln_batch_apply(tmp, 4, mv[:, :, 1], mv[:, :, 0], eps_col,
                               ptmp, pap, dst, dst_off)

            # ============ phase A: kT, v, new_kT ============
            with (
                tc.tile_pool(name="pa", bufs=1) as pa,
                tc.tile_pool(name="pa_wk", bufs=3) as pwk,
            ):
                ctx_f = pa.tile([128, 8, SK], bf16, tag="ctx")
                nc.sync.dma_start(ctx_f[:], r8(d_ctxT))
                bv_row = pa.tile([1, D], f32r, tag="bv_row")
                nc.sync.dma_start(bv_row[:], vec2d("bv").bitcast(f32r))
                bmat_row = pa.tile([1, D], f32r, tag="bmat_row")
                nc.sync.dma_start(bmat_row[:], vec2d("bmat").bitcast(f32r))
                nc.gpsimd.memset(v_aug[:, :, :, 0:64], 0.0)
                nc.gpsimd.memset(v_aug[:, :, :, 0:1], 1.0)
                wv_f = pa.tile([128, 8, D], bf16, tag="wv")
                wm_f = pa.tile([128, 8, D], bf16, tag="wm")

                # kT[ot] = WkT.T @ ctxT + bk, two 512-col halves
                for ot in range(8):
                    wk_c = pwk.tile([128, 8, 128], bf16, tag="wk")
                    nc.sync.dma_start(
                        wk_c[:], r8(d_w["WkT"])[:, :, ot * 128:(ot + 1) * 128]
                    )
                    for sh in range(2):
                        shs = slice(sh * 512, (sh + 1) * 512)
                        ps = pap.tile([128, 512], f32, tag="mm")
                        for i in range(8):
                            nc.tensor.matmul(
                                ps[:], wk_c[:, i, :], ctx_f[:, i, shs],
                                start=(i == 0), stop=(i == 7),
                            )
                        nc.vector.tensor_scalar_add(
                            kT[:, ot, shs], ps[:], bk_sc[:, ot:ot + 1]
                        )
                    if ot == 0:
                        nc.sync.dma_start(wv_f[:], r8(d_w["WvT"]))
                    if ot == 4:
                        nc.sync.dma_start(wm_f[:], r8(d_w["WmatT"]))

                # v rows (bias via rank-1 ones x bv); ones col 0 already set
                for oc in range(2):
                    for tt in range(8):
                        ps = pap.tile([128, 512], f32, tag="mm")
                        for i in range(8):
                            nc.tensor.matmul(
                                ps[:], ctx_f[:, i, tt * 128:(tt + 1) * 128],
                                wv_f[:, i, oc * 512:(oc + 1) * 512],
                                start=(i == 0), stop=False,
                            )
                        nc.tensor.matmul(
                            ps[:], ones_row[:], bv_row[:, oc * 512:(oc + 1) * 512],
                            start=False, stop=True,
                        )
                        nc.vector.tensor_copy(
                            v_aug[:, tt, oc * 8:(oc + 1) * 8, 64:128],
                            ps[:].rearrange("p (h j) -> p h j", j=DH),
                        )

                # new_k: relu(k @ Wmat.T + bmat) -> LN -> transpose -> nkT
                for grp in range(2):
                    tmpk = ptmp.tile([128, 4, 1024], f16, tag="tmp")
                    stats = ptmp.tile([128, 4, 2, 6], f32, tag="stats")
                    mv = ptmp.tile([128, 4, 2], f32, tag="mv")
                    for st in range(4):
                        tt = grp * 4 + st
                        for oc in range(2):
                            ps = pap.tile([128, 512], f32, tag="mm")
                            for i in range(8):
                                nc.tensor.matmul(
                                    ps[:], kT[:, i, tt * 128:(tt + 1) * 128],
                                    wm_f[:, i, oc * 512:(oc + 1) * 512],
                                    start=(i == 0), stop=False,
                                )
                            nc.tensor.matmul(
                                ps[:], ones_row[:],
                                bmat_row[:, oc * 512:(oc + 1) * 512],
                                start=False, stop=True,
                            )
                            nc.scalar.activation(
                                tmpk[:, st, oc * 512:(oc + 1) * 512], ps[:], AF.Relu
                            )
                            nc.vector.bn_stats(
                                stats[:, st, oc, :],
                                tmpk[:, st, oc * 512:(oc + 1) * 512],
                            )
                        nc.vector.bn_aggr(mv[:, st, :], stats[:, st, :, :])
                    ln_group(tmpk, mv, eps_k[:, grp * 4:(grp + 1) * 4],
                             nkT, grp * 512)

            # ============ phase B: q path -> nqT (two s-halves) ============
            pwob_cm = tc.tile_pool(name="p_wobs", bufs=1)
            pwob = pwob_cm.__enter__()
            wobs_f = pwob.tile([128, 8, D], bf16, tag="wobs")
            nc.sync.dma_start(wobs_f[:], r8(d_w["WobsT"]))
            pbpr_cm = tc.tile_pool(name="pb_probs", bufs=1)
            pbpr = pbpr_cm.__enter__()

            def emit_B(sc):
                scs = slice(sc * 512, (sc + 1) * 512)
                hs_c = phs.tile([128, 8, 512], bf16, tag="hs", bufs=1)
                nc.sync.dma_start(hs_c[:], r8(d_hsT)[:, :, scs])
                qT_c = phs.tile([128, 8, 512], bf16, tag="qs")
                for ot in range(8):
                    wq_c = pwq.tile([128, 8, 128], bf16, tag="wq")
                    nc.sync.dma_start(
                        wq_c[:], r8(d_w["WqT"])[:, :, ot * 128:(ot + 1) * 128]
                    )
                    ps = pap.tile([128, 512], f32, tag="mm")
                    for i in range(8):
                        nc.tensor.matmul(
                            ps[:], wq_c[:, i, :], hs_c[:, i, :],
                            start=(i == 0), stop=(i == 7),
                        )
                    nc.vector.tensor_scalar_add(
                        qT_c[:, ot, :], ps[:], bq_sc[:, ot:ot + 1]
                    )
                # probsT = exp(scoresT / 32), bf16 (unnormalized)
                probs = pbpr.tile([128, 8, 512], bf16, tag="probs")
                for tt in range(8):
                    ps = pap.tile([128, 512], f32, tag="mm")
                    for oc in range(8):
                        nc.tensor.matmul(
                            ps[:], kT[:, oc, tt * 128:(tt + 1) * 128],
                            qT_c[:, oc, :],
                            start=(oc == 0), stop=(oc == 7),
                        )
                    nc.scalar.activation(
                        probs[:, tt, :], ps[:], AF.Exp, scale=1.0 / 32.0
                    )
                # column sums S[s]; spread to [s_p, s_c] via DRAM bounce
                psS = pap.tile([128, 512], f32, tag="mm")
                for tt in range(8):
                    nc.tensor.matmul(
                        psS[0:1, :], ones_col_bf[:], probs[:, tt, :],
                        start=(tt == 0), stop=(tt == 7),
                    )
                nc.vector.tensor_copy(S_row[:, scs], psS[0:1, :])
                nc.sync.dma_start(d_rs[scs], S_row[:, scs].bitcast(f32))
                nc.sync.dma_start(
                    Sc[:, sc * 4:(sc + 1) * 4],
                    d_rs[scs].rearrange("(c p) -> p c", p=128),
                )
                # gctxT[o, s] = v.T @ probsT (unnormalized); row-tiled pairs
                gctx = phs.tile([128, 8, 512], bf16, tag="qs")
                for ot in range(8):
                    ps = pap.tile([128, 512], f32, tag="mm")
                    for tt in range(8):
                        for hl in range(2):
                            nc.tensor.matmul(
                                ps[hl * 64:(hl + 1) * 64, :],
                                v_aug[:, tt, 2 * ot + hl, 64:128],
                                probs[:, tt, :],
                                start=(tt == 0), stop=(tt == 7),
                            )
                    nc.vector.tensor_copy(gctx[:, ot, :], ps[:])
                # preq = relu(gctx_raw @ WobsT + S*bobs); LN w/ eps*S^2
                tmpq = ptmp.tile([128, 4, 1024], f16, tag="tmp")
                statq = ptmp.tile([128, 4, 2, 6], f32, tag="stats")
                mvq = ptmp.tile([128, 4, 2], f32, tag="mv")
                for st in range(4):
                    gst = sc * 4 + st
                    for oc in range(2):
                        ps = pap.tile([128, 512], f32, tag="mm")
                        for i in range(8):
                            nc.tensor.matmul(
                                ps[:], gctx[:, i, st * 128:(st + 1) * 128],
                                wobs_f[:, i, oc * 512:(oc + 1) * 512],
                                start=(i == 0), stop=False,
                            )
                        nc.tensor.matmul(
                            ps[:], S_row[:, gst * 128:(gst + 1) * 128],
                            bobs_row[:, oc * 512:(oc + 1) * 512],
                            start=False, stop=True,
                        )
                        nc.scalar.activation(
                            tmpq[:, st, oc * 512:(oc + 1) * 512], ps[:], AF.Relu
                        )
                        nc.vector.bn_stats(
                            statq[:, st, oc, :],
                            tmpq[:, st, oc * 512:(oc + 1) * 512],
                        )
                    nc.vector.bn_aggr(mvq[:, st, :], statq[:, st, :, :])
                # eps' = eps * S^2 for this half
                ecols = Sc[:, sc * 4:(sc + 1) * 4]
                eq = eps_q[:, sc * 4:(sc + 1) * 4]
                nc.vector.tensor_mul(eq, ecols, ecols)
                nc.vector.tensor_scalar_mul(eq, eq, LN_EPS)
                ln_group(tmpq, mvq, eq, nqT, sc * 512)

            emit_B(0)
            emit_B(1)

            # ============ phase D: 16-head MHA over query-halves ============
            def emit_D_half(sh, pdp, pds, psc_pool, po_pool):
                shs = slice(sh * 512, (sh + 1) * 512)
                for hc in range(H // 2):
                    probs = pdp.tile([128, 8, 2, 512], bf16, tag="probs_h")
                    for tt in range(8):
                        ps = psc_pool.tile([128, 2, 512], f32, tag="psc")
                        nc.tensor.matmul(
                            ps[:, 0, :],
                            nkT[0:64, hc, tt * 128:(tt + 1) * 128],
                            nqT[0:64, hc, shs], start=True, stop=True,
                        )
                        nc.tensor.matmul(
                            ps[:, 1, :],
                            nkT[64:128, hc, tt * 128:(tt + 1) * 128],
                            nqT[64:128, hc, shs], start=True, stop=True,
                        )
                        nc.scalar.activation(
                            probs[:, tt, :, :], ps[:], AF.Exp, scale=1.0 / 8.0
                        )
                    for hl in range(2):
                        h = 2 * hc + hl
                        ps = po_pool.tile([128, 512], f32, tag="po")
                        for tt in range(8):
                            nc.tensor.matmul(
                                ps[:], v_aug[:, tt, h, :],
                                probs[:, tt, hl, :],
                                start=(tt == 0), stop=(tt == 7),
                            )
                        # sums on partition 0; 1/S straight off PSUM
                        rinv = pds.tile([1, 512], f32, tag="rinv")
                        nc.vector.reciprocal_approx_fast(rinv[:], ps[0:1, :])
                        nc.sync.dma_start(d_rinv[h:h + 1, shs], rinv[:])
                        rbc = pds.tile([128, 512], f32, tag="rbc")
                        nc.sync.dma_start(
                            rbc[64:128, :],
                            d_rinv[h:h + 1, shs].to_broadcast([64, 512]),
                        )
                        outF = pds.tile([128, 512], f32, tag="outF")
                        nc.vector.tensor_mul(
                            outF[64:128, :], ps[64:128, :], rbc[64:128, :]
                        )
                        nc.sync.dma_start(
                            d_outT[h * DH:(h + 1) * DH, shs], outF[64:128, :]
                        )

            # D0 (emitted after B1 so B1 keeps PE priority; D0's exps fill
            # ACT while B1 owns the PE)
            with (
                tc.tile_pool(name="pd_probs0", bufs=2) as pdp0,
                tc.tile_pool(name="pd_st0", bufs=2) as pds0,
                tc.tile_pool(name="ps_sc0", bufs=1, space="PSUM") as psc0,
                tc.tile_pool(name="ps_po0", bufs=2, space="PSUM") as po0,
            ):
                emit_D_half(0, pdp0, pds0, psc0, po0)

            # close phase-B pools before D1 so D1 gets wide PSUM pools
            pbpr_cm.__exit__(None, None, None)
            pwob_cm.__exit__(None, None, None)
            pmm_cm.__exit__(None, None, None)
            ptmp_cm.__exit__(None, None, None)
            pwq_cm.__exit__(None, None, None)
            phs_cm.__exit__(None, None, None)
            pab_cm.__exit__(None, None, None)

            with (
                tc.tile_pool(name="pd_probs1", bufs=2) as pdp1,
                tc.tile_pool(name="pd_st1", bufs=4) as pds1,
                tc.tile_pool(name="ps_sc1", bufs=2, space="PSUM") as psc1,
                tc.tile_pool(name="ps_po1", bufs=3, space="PSUM") as po1,
            ):
                emit_D_half(1, pdp1, pds1, psc1, po1)

    nc.compile()
    return nc


def _prep_in_maps(inputs):
    import ml_dtypes

    bf = ml_dtypes.bfloat16
    w = {
        "WqT": np.ascontiguousarray(np.asarray(inputs["Wq"]).T).astype(bf),
        "WkT": np.ascontiguousarray(np.asarray(inputs["Wk"]).T).astype(bf),
        "WvT": np.ascontiguousarray(np.asarray(inputs["Wv"]).T).astype(bf),
        "WobsT": np.ascontiguousarray(np.asarray(inputs["Wobs"]).T).astype(bf),
        "WmatT": np.ascontiguousarray(np.asarray(inputs["Wmat"]).T).astype(bf),
    }
    vecs = {
        k: np.ascontiguousarray(np.asarray(inputs[k], dtype=np.float32))
        for k in ("bq", "bk", "bv", "bobs", "bmat", "ln_g", "ln_b")
    }
    hs = np.asarray(inputs["hidden_states"])
    ctx = np.asarray(inputs["context"])
    in_maps = []
    for b in range(N_CORES):
        m = {
            "hsT": np.ascontiguousarray(hs[b].T).astype(bf),
            "ctxT": np.ascontiguousarray(ctx[b].T).astype(bf),
        }
        m.update(w)
        m.update(vecs)
        in_maps.append(m)
    return in_maps


def kernel(hidden_states, context, Wq, bq, Wk, bk, Wv, bv,
           Wobs, bobs, Wmat, bmat, ln_g, ln_b):
    from concourse import bass_utils

    if "nc" not in _CACHE:
        _CACHE["nc"] = _build()
    nc = _CACHE["nc"]

    in_maps = _prep_in_maps(dict(
        hidden_states=hidden_states, context=context, Wq=Wq, Wk=Wk, Wv=Wv,
        Wobs=Wobs, Wmat=Wmat, bq=bq, bk=bk, bv=bv, bobs=bobs, bmat=bmat,
        ln_g=ln_g, ln_b=ln_b,
    ))
    res = bass_utils.run_bass_kernel_spmd(nc, in_maps, core_ids=list(range(N_CORES)))
    out = np.stack([res.results[b]["outT"].T for b in range(N_CORES)], axis=0)
    return out.astype(np.float32)


# revision 17
# speedup vs baseline: 1.1077x; 1.1077x over previous
"""GuidedAttention Trainium2 kernel — batch-parallel over 8 NeuronCores.

Per core (one batch element, SQ=SK=D=1024, H=16, DH=64):
  q = hs @ Wq.T + bq ; k = ctx @ Wk.T + bk ; v = ctx @ Wv.T + bv
  attn1 = softmax(q k^T / 32) ; gctx = attn1 @ v
  new_q = LN(relu(gctx @ Wobs.T + bobs)) ; new_k = LN(relu(k @ Wmat.T + bmat))
  out = MHA(new_q, new_k, v)  (16 heads of 64)

v3 design (over the v2 all-SBUF bf16 kernel):
  * LN block transposes are ordinary matmuls against a f16 identity
    (stationary = the normalized fp16 block, FWL-eligible) instead of
    is_transpose PE ops — ~2.5x cheaper on PE; affine applied on 4
    batched blocks per PSUM bank.
  * ctx resident in SBUF; Wk streamed exactly once (v2 re-streamed it
    4x); kT emitted as two N=512 column halves.
  * MHA runs as two query-halves D0/D1.  D0 only depends on the first
    LN half, so the tile scheduler overlaps D0's exp stream (the ACT
    bottleneck) with phase B1's PE-bound work.
  * v_aug carries its softmax-sum ones column FIRST, so the per-head
    denominators land on PSUM partition 0 where reciprocal_approx_fast
    can read them directly — no DRAM bounces in the MHA tail.
Output is produced transposed [D, SQ]; the host transposes it back.
"""

import numpy as np

B, SQ, SK, D, H = 8, 1024, 1024, 1024, 16
DH = D // H
LN_EPS = 1e-5
N_CORES = 8

_CACHE = {}


def _build():
    import concourse.mybir as mybir
    import concourse.tile as tile
    from concourse import bacc
    from concourse.masks import make_identity

    f32 = mybir.dt.float32
    f32r = mybir.dt.float32r
    bf16 = mybir.dt.bfloat16
    f16 = mybir.dt.float16
    AF = mybir.ActivationFunctionType
    ALU = mybir.AluOpType

    nc = bacc.Bacc(None, target_bir_lowering=False)

    d_hsT = nc.dram_tensor("hsT", [D, SQ], bf16, kind="ExternalInput")
    d_ctxT = nc.dram_tensor("ctxT", [D, SK], bf16, kind="ExternalInput")
    d_w = {
        n: nc.dram_tensor(n, [D, D], bf16, kind="ExternalInput")
        for n in ("WqT", "WkT", "WvT", "WobsT", "WmatT")
    }
    d_b = {
        n: nc.dram_tensor(n, [D], f32, kind="ExternalInput")
        for n in ("bq", "bk", "bv", "bobs", "bmat", "ln_g", "ln_b")
    }
    d_outT = nc.dram_tensor("outT", [D, SQ], f32, kind="ExternalOutput")
    d_rs = nc.dram_tensor("rs_scratch", [SQ], f32, kind="Internal")
    d_rinv = nc.dram_tensor("rinv_scratch", [H, SQ], f32, kind="Internal")

    def r8(ap):  # [(c p), x] -> [p, c, x]
        return ap.rearrange("(c p) x -> p c x", p=128)

    def vec2d(name):  # [D] -> [1, D] AP
        return d_b[name][:].rearrange("(a d) -> a d", a=1)

    with tile.TileContext(nc) as tc:
        with tc.tile_pool(name="persist", bufs=1) as pp:
            # ---- constants / persistent tiles ----
            v_aug = pp.tile([128, 8, H, 128], bf16, tag="vaug")
            nqT = pp.tile([128, 8, SQ], bf16, tag="nqT")
            nkT = pp.tile([128, 8, SK], bf16, tag="nkT")
            ident = pp.tile([128, 128], f32, tag="ident")
            make_identity(nc, ident[:])
            ident_h = pp.tile([128, 128], f16, tag="ident_h")
            nc.vector.tensor_copy(ident_h[:], ident[:])
            ones_row = pp.tile([1, 128], f32r, tag="ones_row")
            d_ones = nc.inline_tensor(np.ones((1, 128), np.float32), name="ones_const")
            nc.sync.dma_start(ones_row[:], d_ones[:].bitcast(f32r))
            ones_col_bf = pp.tile([128, 1], bf16, tag="ones_col")
            nc.vector.memset(ones_col_bf[:], 1.0)
            bq_sc = pp.tile([128, 8], f32, tag="bq_sc")
            bk_sc = pp.tile([128, 8], f32, tag="bk_sc")
            nc.sync.dma_start(bq_sc[:], d_b["bq"][:].rearrange("(c p) -> p c", p=128))
            nc.sync.dma_start(bk_sc[:], d_b["bk"][:].rearrange("(c p) -> p c", p=128))
            g_sc = pp.tile([128, 8], f32, tag="g_sc")
            b_sc = pp.tile([128, 8], f32, tag="b_sc")
            nc.sync.dma_start(g_sc[:], d_b["ln_g"][:].rearrange("(c p) -> p c", p=128))
            nc.sync.dma_start(b_sc[:], d_b["ln_b"][:].rearrange("(c p) -> p c", p=128))
            bobs_row = pp.tile([1, D], f32r, tag="bobs_row")
            nc.sync.dma_start(bobs_row[:], vec2d("bobs").bitcast(f32r))
            # guide softmax column sums, [s_p, s_c] layout + derived eps'
            Sc = pp.tile([128, 8], f32, tag="Sc")
            eps_q = pp.tile([128, 8], f32, tag="eps_q")
            S_row = pp.tile([1, SQ], f32r, tag="S_row")

            def ln_batch_apply(tmp, n_str, var_all, mean_all, eps_col, tpool,
                               mm_pool, dst, dst_off):
                """tmp [128, n_str, 1024] fp16 relu'd rows (s on partitions),
                per-stripe stats APs [128, n_str].  Normalize in place,
                transpose 128x128 blocks via identity-matmul, then
                per-partition affine (g, b) into dst[:, ot, dst_off...]."""
                lnv = tpool.tile([128, n_str], f32, tag="lnv")
                nc.vector.tensor_tensor(lnv[:], var_all, eps_col, ALU.add)
                std = tpool.tile([128, n_str], f32, tag="std")
                nc.scalar.activation(std[:], lnv[:], AF.Sqrt)
                rstd = tpool.tile([128, n_str], f32, tag="rstd")
                nc.vector.reciprocal(rstd[:], std[:])
                negmr = tpool.tile([128, n_str], f32, tag="negmr")
                nc.vector.tensor_mul(negmr[:], mean_all, rstd[:])
                nc.vector.tensor_scalar_mul(negmr[:], negmr[:], -1.0)
                for i in range(n_str):
                    nc.vector.tensor_scalar(
                        tmp[:, i, :], tmp[:, i, :],
                        rstd[:, i:i + 1], negmr[:, i:i + 1],
                        ALU.mult, ALU.add,
                    )
                # transpose: out[d, s] = sum_s' tmp[s', d] * I[s', s]
                for ot in range(8):
                    pst = mm_pool.tile([128, 512], f32, tag="mm")
                    for i in range(n_str):
                        nc.tensor.matmul(
                            pst[:, i * 128:(i + 1) * 128],
                            tmp[:, i, ot * 128:(ot + 1) * 128],
                            ident_h[:], start=True, stop=True,
                        )
                    nc.vector.tensor_scalar(
                        dst[:, ot, dst_off:dst_off + n_str * 128],
                        pst[:], g_sc[:, ot:ot + 1], b_sc[:, ot:ot + 1],
                        ALU.mult, ALU.add,
                    )

            # ============ shared pools across A+B ============
            pab_cm = tc.tile_pool(name="pab", bufs=1)
            pab = pab_cm.__enter__()
            kT = pab.tile([128, 8, SK], bf16, tag="kT")
            eps_k = pab.tile([128, 8], f32, tag="eps_k")
            nc.vector.memset(eps_k[:], LN_EPS)
            phs_cm = tc.tile_pool(name="p_hs", bufs=2)
            phs = phs_cm.__enter__()
            pwq_cm = tc.tile_pool(name="p_wq", bufs=3)
            pwq = pwq_cm.__enter__()
            ptmp_cm = tc.tile_pool(name="p_tmp", bufs=2)
            ptmp = ptmp_cm.__enter__()
            pmm_cm = tc.tile_pool(name="ps_mm", bufs=2, space="PSUM")
            pap = pmm_cm.__enter__()

            def ln_group(tmp, mv, eps_col, dst, dst_off):
                ln_batch_apply(tmp, 4, mv[:, :, 1], mv[:, :, 0], eps_col,
                               ptmp, pap, dst, dst_off)

            # ============ phase A: kT, v, new_kT ============
            with (
                tc.tile_pool(name="pa", bufs=1) as pa,
                tc.tile_pool(name="pa_wk", bufs=3) as pwk,
            ):
                ctx_f = pa.tile([128, 8, SK], bf16, tag="ctx")
                nc.sync.dma_start(ctx_f[:], r8(d_ctxT))
                bv_row = pa.tile([1, D], f32r, tag="bv_row")
                nc.sync.dma_start(bv_row[:], vec2d("bv").bitcast(f32r))
                bmat_row = pa.tile([1, D], f32r, tag="bmat_row")
                nc.sync.dma_start(bmat_row[:], vec2d("bmat").bitcast(f32r))
                nc.gpsimd.memset(v_aug[:, :, :, 0:64], 0.0)
                nc.gpsimd.memset(v_aug[:, :, :, 0:1], 1.0)
                wv_f = pa.tile([128, 8, D], bf16, tag="wv")
                wm_f = pa.tile([128, 8, D], bf16, tag="wm")

                # kT[ot] = WkT.T @ ctxT + bk, two 512-col halves
                for ot in range(8):
                    wk_c = pwk.tile([128, 8, 128], bf16, tag="wk")
                    nc.sync.dma_start(
                        wk_c[:], r8(d_w["WkT"])[:, :, ot * 128:(ot + 1) * 128]
                    )
                    for sh in range(2):
                        shs = slice(sh * 512, (sh + 1) * 512)
                        ps = pap.tile([128, 512], f32, tag="mm")
                        for i in range(8):
                            nc.tensor.matmul(
                                ps[:], wk_c[:, i, :], ctx_f[:, i, shs],
                                start=(i == 0), stop=(i == 7),
                            )
                        nc.vector.tensor_scalar_add(
                            kT[:, ot, shs], ps[:], bk_sc[:, ot:ot + 1]
                        )
                    if ot == 0:
                        nc.sync.dma_start(wv_f[:], r8(d_w["WvT"]))
                    if ot == 4:
                        nc.sync.dma_start(wm_f[:], r8(d_w["WmatT"]))

                # v rows (bias via rank-1 ones x bv); ones col 0 already set
                for oc in range(2):
                    for tt in range(8):
                        ps = pap.tile([128, 512], f32, tag="mm")
                        for i in range(8):
                            nc.tensor.matmul(
                                ps[:], ctx_f[:, i, tt * 128:(tt + 1) * 128],
                                wv_f[:, i, oc * 512:(oc + 1) * 512],
                                start=(i == 0), stop=False,
                            )
                        nc.tensor.matmul(
                            ps[:], ones_row[:], bv_row[:, oc * 512:(oc + 1) * 512],
                            start=False, stop=True,
                        )
                        nc.vector.tensor_copy(
                            v_aug[:, tt, oc * 8:(oc + 1) * 8, 64:128],
                            ps[:].rearrange("p (h j) -> p h j", j=DH),
                        )

                # new_k: relu(k @ Wmat.T + bmat) -> LN -> transpose -> nkT
                for grp in range(2):
                    tmpk = ptmp.tile([128, 4, 1024], f16, tag="tmp")
                    stats = ptmp.tile([128, 4, 2, 6], f32, tag="stats")
                    mv = ptmp.tile([128, 4, 2], f32, tag="mv")
                    for st in range(4):
                        tt = grp * 4 + st
                        for oc in range(2):
                            ps = pap.tile([128, 512], f32, tag="mm")
                            for i in range(8):
                                nc.tensor.matmul(
                                    ps[:], kT[:, i, tt * 128:(tt + 1) * 128],
                                    wm_f[:, i, oc * 512:(oc + 1) * 512],
                                    start=(i == 0), stop=False,
                                )
                            nc.tensor.matmul(
                                ps[:], ones_row[:],
                                bmat_row[:, oc * 512:(oc + 1) * 512],
                                start=False, stop=True,
                            )
                            nc.scalar.activation(
                                tmpk[:, st, oc * 512:(oc + 1) * 512], ps[:], AF.Relu
                            )
                            nc.vector.bn_stats(
                                stats[:, st, oc, :],
                                tmpk[:, st, oc * 512:(oc + 1) * 512],
                            )
                        nc.vector.bn_aggr(mv[:, st, :], stats[:, st, :, :])
                    ln_group(tmpk, mv, eps_k[:, grp * 4:(grp + 1) * 4],
                             nkT, grp * 512)

            # ============ phase B: q path -> nqT (two s-halves) ============
            pwob_cm = tc.tile_pool(name="p_wobs", bufs=1)
            pwob = pwob_cm.__enter__()
            wobs_f = pwob.tile([128, 8, D], bf16, tag="wobs")
            nc.sync.dma_start(wobs_f[:], r8(d_w["WobsT"]))
            pbpr_cm = tc.tile_pool(name="pb_probs", bufs=1)
            pbpr = pbpr_cm.__enter__()

            def emit_B(sc):  # generator: yields at stage boundaries
                scs = slice(sc * 512, (sc + 1) * 512)
                hs_c = phs.tile([128, 8, 512], bf16, tag="hs", bufs=1)
                nc.sync.dma_start(hs_c[:], r8(d_hsT)[:, :, scs])
                qT_c = phs.tile([128, 8, 512], bf16, tag="qs")
                for ot in range(8):
                    wq_c = pwq.tile([128, 8, 128], bf16, tag="wq")
                    nc.sync.dma_start(
                        wq_c[:], r8(d_w["WqT"])[:, :, ot * 128:(ot + 1) * 128]
                    )
                    ps = pap.tile([128, 512], f32, tag="mm")
                    for i in range(8):
                        nc.tensor.matmul(
                            ps[:], wq_c[:, i, :], hs_c[:, i, :],
                            start=(i == 0), stop=(i == 7),
                        )
                    nc.vector.tensor_scalar_add(
                        qT_c[:, ot, :], ps[:], bq_sc[:, ot:ot + 1]
                    )
                yield
                # probsT = exp(scoresT / 32), bf16 (unnormalized)
                probs = pbpr.tile([128, 8, 512], bf16, tag="probs")
                for tt in range(8):
                    ps = pap.tile([128, 512], f32, tag="mm")
                    for oc in range(8):
                        nc.tensor.matmul(
                            ps[:], kT[:, oc, tt * 128:(tt + 1) * 128],
                            qT_c[:, oc, :],
                            start=(oc == 0), stop=(oc == 7),
                        )
                    nc.scalar.activation(
                        probs[:, tt, :], ps[:], AF.Exp, scale=1.0 / 32.0
                    )
                yield
                # column sums S[s]; spread to [s_p, s_c] via DRAM bounce
                psS = pap.tile([128, 512], f32, tag="mm")
                for tt in range(8):
                    nc.tensor.matmul(
                        psS[0:1, :], ones_col_bf[:], probs[:, tt, :],
                        start=(tt == 0), stop=(tt == 7),
                    )
                nc.vector.tensor_copy(S_row[:, scs], psS[0:1, :])
                nc.sync.dma_start(d_rs[scs], S_row[:, scs].bitcast(f32))
                nc.sync.dma_start(
                    Sc[:, sc * 4:(sc + 1) * 4],
                    d_rs[scs].rearrange("(c p) -> p c", p=128),
                )
                # gctxT[o, s] = v.T @ probsT (unnormalized); row-tiled pairs
                gctx = phs.tile([128, 8, 512], bf16, tag="qs")
                for ot in range(8):
                    ps = pap.tile([128, 512], f32, tag="mm")
                    for tt in range(8):
                        for hl in range(2):
                            nc.tensor.matmul(
                                ps[hl * 64:(hl + 1) * 64, :],
                                v_aug[:, tt, 2 * ot + hl, 64:128],
                                probs[:, tt, :],
                                start=(tt == 0), stop=(tt == 7),
                            )
                    nc.vector.tensor_copy(gctx[:, ot, :], ps[:])
                yield
                # preq = relu(gctx_raw @ WobsT + S*bobs); LN w/ eps*S^2
                tmpq = ptmp.tile([128, 4, 1024], f16, tag="tmp")
                statq = ptmp.tile([128, 4, 2, 6], f32, tag="stats")
                mvq = ptmp.tile([128, 4, 2], f32, tag="mv")
                for st in range(4):
                    gst = sc * 4 + st
                    for oc in range(2):
                        ps = pap.tile([128, 512], f32, tag="mm")
                        for i in range(8):
                            nc.tensor.matmul(
                                ps[:], gctx[:, i, st * 128:(st + 1) * 128],
                                wobs_f[:, i, oc * 512:(oc + 1) * 512],
                                start=(i == 0), stop=False,
                            )
                        nc.tensor.matmul(
                            ps[:], S_row[:, gst * 128:(gst + 1) * 128],
                            bobs_row[:, oc * 512:(oc + 1) * 512],
                            start=False, stop=True,
                        )
                        nc.scalar.activation(
                            tmpq[:, st, oc * 512:(oc + 1) * 512], ps[:], AF.Relu
                        )
                        nc.vector.bn_stats(
                            statq[:, st, oc, :],
                            tmpq[:, st, oc * 512:(oc + 1) * 512],
                        )
                    nc.vector.bn_aggr(mvq[:, st, :], statq[:, st, :, :])
                yield
                # eps' = eps * S^2 for this half
                ecols = Sc[:, sc * 4:(sc + 1) * 4]
                eq = eps_q[:, sc * 4:(sc + 1) * 4]
                nc.vector.tensor_mul(eq, ecols, ecols)
                nc.vector.tensor_scalar_mul(eq, eq, LN_EPS)
                ln_group(tmpq, mvq, eq, nqT, sc * 512)

            for _ in emit_B(0):
                pass

            # ============ phase D: 16-head MHA over query-halves ============
            def emit_D_half(sh, pdp, pds, psc_pool, po_pool):  # yields per pair
                shs = slice(sh * 512, (sh + 1) * 512)
                for hc in range(H // 2):
                    probs = pdp.tile([128, 8, 2, 512], bf16, tag="probs_h")
                    for tt in range(8):
                        ps = psc_pool.tile([128, 2, 512], f32, tag="psc")
                        nc.tensor.matmul(
                            ps[:, 0, :],
                            nkT[0:64, hc, tt * 128:(tt + 1) * 128],
                            nqT[0:64, hc, shs], start=True, stop=True,
                        )
                        nc.tensor.matmul(
                            ps[:, 1, :],
                            nkT[64:128, hc, tt * 128:(tt + 1) * 128],
                            nqT[64:128, hc, shs], start=True, stop=True,
                        )
                        nc.scalar.activation(
                            probs[:, tt, :, :], ps[:], AF.Exp, scale=1.0 / 8.0
                        )
                    for hl in range(2):
                        h = 2 * hc + hl
                        ps = po_pool.tile([128, 512], f32, tag="po")
                        for tt in range(8):
                            nc.tensor.matmul(
                                ps[:], v_aug[:, tt, h, :],
                                probs[:, tt, hl, :],
                                start=(tt == 0), stop=(tt == 7),
                            )
                        # sums on partition 0; 1/S straight off PSUM
                        rinv = pds.tile([1, 512], f32, tag="rinv")
                        nc.vector.reciprocal_approx_fast(rinv[:], ps[0:1, :])
                        nc.sync.dma_start(d_rinv[h:h + 1, shs], rinv[:])
                        rbc = pds.tile([128, 512], f32, tag="rbc")
                        nc.sync.dma_start(
                            rbc[64:128, :],
                            d_rinv[h:h + 1, shs].to_broadcast([64, 512]),
                        )
                        outF = pds.tile([128, 512], f32, tag="outF")
                        nc.vector.tensor_mul(
                            outF[64:128, :], ps[64:128, :], rbc[64:128, :]
                        )
                        nc.sync.dma_start(
                            d_outT[h * DH:(h + 1) * DH, shs], outF[64:128, :]
                        )
                    yield

            # interleave B1 stage emission with D0 pairs so the scheduler
            # keeps ACT fed with D0 exps while B1 owns most of the PE
            with (
                tc.tile_pool(name="pd_probs0", bufs=2) as pdp0,
                tc.tile_pool(name="pd_st0", bufs=2) as pds0,
                tc.tile_pool(name="ps_sc0", bufs=2, space="PSUM") as psc0,
                tc.tile_pool(name="ps_po0", bufs=2, space="PSUM") as po0,
            ):
                b1 = emit_B(1)
                d0 = emit_D_half(0, pdp0, pds0, psc0, po0)
                next(b1, None)  # qT
                next(b1, None)  # guide probs
                for _ in range(2):
                    next(d0, None)
                next(b1, None)  # colsum + gctx
                next(d0, None)
                next(b1, None)  # preq
                next(d0, None)
                next(b1, None)  # LN tail
                for _ in d0:
                    pass

            # close phase-B pools before D1 so D1 gets wide PSUM pools
            pbpr_cm.__exit__(None, None, None)
            pwob_cm.__exit__(None, None, None)
            pmm_cm.__exit__(None, None, None)
            ptmp_cm.__exit__(None, None, None)
            pwq_cm.__exit__(None, None, None)
            phs_cm.__exit__(None, None, None)
            pab_cm.__exit__(None, None, None)

            with (
                tc.tile_pool(name="pd_probs1", bufs=2) as pdp1,
                tc.tile_pool(name="pd_st1", bufs=4) as pds1,
                tc.tile_pool(name="ps_sc1", bufs=2, space="PSUM") as psc1,
                tc.tile_pool(name="ps_po1", bufs=4, space="PSUM") as po1,
            ):
                for _ in emit_D_half(1, pdp1, pds1, psc1, po1):
                    pass

    nc.compile()
    return nc


def _prep_in_maps(inputs):
    import ml_dtypes

    bf = ml_dtypes.bfloat16
    w = {
        "WqT": np.ascontiguousarray(np.asarray(inputs["Wq"]).T).astype(bf),
        "WkT": np.ascontiguousarray(np.asarray(inputs["Wk"]).T).astype(bf),
        "WvT": np.ascontiguousarray(np.asarray(inputs["Wv"]).T).astype(bf),
        "WobsT": np.ascontiguousarray(np.asarray(inputs["Wobs"]).T).astype(bf),
        "WmatT": np.ascontiguousarray(np.asarray(inputs["Wmat"]).T).astype(bf),
    }
    vecs = {
        k: np.ascontiguousarray(np.asarray(inputs[k], dtype=np.float32))
        for k in ("bq", "bk", "bv", "bobs", "bmat", "ln_g", "ln_b")
    }
    hs = np.asarray(inputs["hidden_states"])
    ctx = np.asarray(inputs["context"])
    in_maps = []
    for b in range(N_CORES):
        m = {
            "hsT": np.ascontiguousarray(hs[b].T).astype(bf),
            "ctxT": np.ascontiguousarray(ctx[b].T).astype(bf),
        }
        m.update(w)
        m.update(vecs)
        in_maps.append(m)
    return in_maps


def kernel(hidden_states, context, Wq, bq, Wk, bk, Wv, bv,
           Wobs, bobs, Wmat, bmat, ln_g, ln_b):
    from concourse import bass_utils

    if "nc" not in _CACHE:
        _CACHE["nc"] = _build()
    nc = _CACHE["nc"]

    in_maps = _prep_in_maps(dict(
        hidden_states=hidden_states, context=context, Wq=Wq, Wk=Wk, Wv=Wv,
        Wobs=Wobs, Wmat=Wmat, bq=bq, bk=bk, bv=bv, bobs=bobs, bmat=bmat,
        ln_g=ln_g, ln_b=ln_b,
    ))
    res = bass_utils.run_bass_kernel_spmd(nc, in_maps, core_ids=list(range(N_CORES)))
    out = np.stack([res.results[b]["outT"].T for b in range(N_CORES)], axis=0)
    return out.astype(np.float32)


# revision 18
# speedup vs baseline: 1.1438x; 1.0326x over previous
"""GuidedAttention Trainium2 kernel — batch-parallel over 8 NeuronCores.

Per core (one batch element, SQ=SK=D=1024, H=16, DH=64):
  q = hs @ Wq.T + bq ; k = ctx @ Wk.T + bk ; v = ctx @ Wv.T + bv
  attn1 = softmax(q k^T / 32) ; gctx = attn1 @ v
  new_q = LN(relu(gctx @ Wobs.T + bobs)) ; new_k = LN(relu(k @ Wmat.T + bmat))
  out = MHA(new_q, new_k, v)  (16 heads of 64)

v3 design (over the v2 all-SBUF bf16 kernel):
  * LN block transposes are ordinary matmuls against a f16 identity
    (stationary = the normalized fp16 block, FWL-eligible) instead of
    is_transpose PE ops — ~2.5x cheaper on PE; affine applied on 4
    batched blocks per PSUM bank.
  * ctx resident in SBUF; Wk streamed exactly once (v2 re-streamed it
    4x); kT emitted as two N=512 column halves.
  * MHA runs as two query-halves D0/D1.  D0 only depends on the first
    LN half, so the tile scheduler overlaps D0's exp stream (the ACT
    bottleneck) with phase B1's PE-bound work.
  * v_aug carries its softmax-sum ones column FIRST, so the per-head
    denominators land on PSUM partition 0 where reciprocal_approx_fast
    can read them directly — no DRAM bounces in the MHA tail.
Output is produced transposed [D, SQ]; the host transposes it back.
"""

import numpy as np

B, SQ, SK, D, H = 8, 1024, 1024, 1024, 16
DH = D // H
LN_EPS = 1e-5
N_CORES = 8

_CACHE = {}


def _build():
    import concourse.mybir as mybir
    import concourse.tile as tile
    from concourse import bacc
    from concourse.masks import make_identity

    f32 = mybir.dt.float32
    f32r = mybir.dt.float32r
    bf16 = mybir.dt.bfloat16
    f16 = mybir.dt.float16
    AF = mybir.ActivationFunctionType
    ALU = mybir.AluOpType

    nc = bacc.Bacc(None, target_bir_lowering=False)

    d_hsT = nc.dram_tensor("hsT", [D, SQ], bf16, kind="ExternalInput")
    d_ctxT = nc.dram_tensor("ctxT", [D, SK], bf16, kind="ExternalInput")
    d_w = {
        n: nc.dram_tensor(n, [D, D], bf16, kind="ExternalInput")
        for n in ("WqT", "WkT", "WvT", "WobsT", "WmatT")
    }
    d_b = {
        n: nc.dram_tensor(n, [D], f32, kind="ExternalInput")
        for n in ("bq", "bk", "bv", "bobs", "bmat", "ln_g", "ln_b")
    }
    d_outT = nc.dram_tensor("outT", [D, SQ], f32, kind="ExternalOutput")
    d_rs = nc.dram_tensor("rs_scratch", [SQ], f32, kind="Internal")
    d_rinv = nc.dram_tensor("rinv_scratch", [H, SQ], f32, kind="Internal")

    def r8(ap):  # [(c p), x] -> [p, c, x]
        return ap.rearrange("(c p) x -> p c x", p=128)

    def vec2d(name):  # [D] -> [1, D] AP
        return d_b[name][:].rearrange("(a d) -> a d", a=1)

    with tile.TileContext(nc) as tc:
        with tc.tile_pool(name="persist", bufs=1) as pp:
            # ---- constants / persistent tiles ----
            v_aug = pp.tile([128, 8, H, 128], bf16, tag="vaug")
            nqT = pp.tile([128, 8, SQ], bf16, tag="nqT")
            nkT = pp.tile([128, 8, SK], bf16, tag="nkT")
            ident = pp.tile([128, 128], f32, tag="ident")
            make_identity(nc, ident[:])
            ident_h = pp.tile([128, 128], f16, tag="ident_h")
            nc.vector.tensor_copy(ident_h[:], ident[:])
            ones_row = pp.tile([1, 128], f32r, tag="ones_row")
            d_ones = nc.inline_tensor(np.ones((1, 128), np.float32), name="ones_const")
            nc.sync.dma_start(ones_row[:], d_ones[:].bitcast(f32r))
            ones_col_bf = pp.tile([128, 1], bf16, tag="ones_col")
            nc.vector.memset(ones_col_bf[:], 1.0)
            bq_sc = pp.tile([128, 8], f32, tag="bq_sc")
            bk_sc = pp.tile([128, 8], f32, tag="bk_sc")
            nc.sync.dma_start(bq_sc[:], d_b["bq"][:].rearrange("(c p) -> p c", p=128))
            nc.sync.dma_start(bk_sc[:], d_b["bk"][:].rearrange("(c p) -> p c", p=128))
            g_sc = pp.tile([128, 8], f32, tag="g_sc")
            b_sc = pp.tile([128, 8], f32, tag="b_sc")
            nc.sync.dma_start(g_sc[:], d_b["ln_g"][:].rearrange("(c p) -> p c", p=128))
            nc.sync.dma_start(b_sc[:], d_b["ln_b"][:].rearrange("(c p) -> p c", p=128))
            bobs_row = pp.tile([1, D], f32r, tag="bobs_row")
            nc.sync.dma_start(bobs_row[:], vec2d("bobs").bitcast(f32r))
            # guide softmax column sums, [s_p, s_c] layout + derived eps'
            Sc = pp.tile([128, 8], f32, tag="Sc")
            eps_q = pp.tile([128, 8], f32, tag="eps_q")
            S_row = pp.tile([1, SQ], f32r, tag="S_row")

            def ln_batch_apply(tmp, n_str, var_all, mean_all, eps_col, tpool,
                               mm_pool, dst, dst_off):
                """tmp [128, n_str, 1024] fp16 relu'd rows (s on partitions),
                per-stripe stats APs [128, n_str].  Normalize in place,
                transpose 128x128 blocks via identity-matmul, then
                per-partition affine (g, b) into dst[:, ot, dst_off...]."""
                lnv = tpool.tile([128, n_str], f32, tag="lnv")
                nc.vector.tensor_tensor(lnv[:], var_all, eps_col, ALU.add)
                std = tpool.tile([128, n_str], f32, tag="std")
                nc.scalar.activation(std[:], lnv[:], AF.Sqrt)
                rstd = tpool.tile([128, n_str], f32, tag="rstd")
                nc.vector.reciprocal(rstd[:], std[:])
                negmr = tpool.tile([128, n_str], f32, tag="negmr")
                nc.vector.tensor_mul(negmr[:], mean_all, rstd[:])
                nc.vector.tensor_scalar_mul(negmr[:], negmr[:], -1.0)
                for i in range(n_str):
                    nc.vector.tensor_scalar(
                        tmp[:, i, :], tmp[:, i, :],
                        rstd[:, i:i + 1], negmr[:, i:i + 1],
                        ALU.mult, ALU.add,
                    )
                # transpose: out[d, s] = sum_s' tmp[s', d] * I[s', s]
                for ot in range(8):
                    pst = mm_pool.tile([128, 512], f32, tag="mm")
                    for i in range(n_str):
                        nc.tensor.matmul(
                            pst[:, i * 128:(i + 1) * 128],
                            tmp[:, i, ot * 128:(ot + 1) * 128],
                            ident_h[:], start=True, stop=True,
                        )
                    nc.vector.tensor_scalar(
                        dst[:, ot, dst_off:dst_off + n_str * 128],
                        pst[:], g_sc[:, ot:ot + 1], b_sc[:, ot:ot + 1],
                        ALU.mult, ALU.add,
                    )

            # ============ shared pools across A+B ============
            pab_cm = tc.tile_pool(name="pab", bufs=1)
            pab = pab_cm.__enter__()
            kT = pab.tile([128, 8, SK], bf16, tag="kT")
            eps_k = pab.tile([128, 8], f32, tag="eps_k")
            nc.vector.memset(eps_k[:], LN_EPS)
            phs_cm = tc.tile_pool(name="p_hs", bufs=2)
            phs = phs_cm.__enter__()
            pwq_cm = tc.tile_pool(name="p_wq", bufs=3)
            pwq = pwq_cm.__enter__()
            ptmp_cm = tc.tile_pool(name="p_tmp", bufs=2)
            ptmp = ptmp_cm.__enter__()
            pmm_cm = tc.tile_pool(name="ps_mm", bufs=2, space="PSUM")
            pap = pmm_cm.__enter__()

            def ln_group(tmp, mv, eps_col, dst, dst_off):
                ln_batch_apply(tmp, 4, mv[:, :, 1], mv[:, :, 0], eps_col,
                               ptmp, pap, dst, dst_off)

            # ============ phase A: kT, v, new_kT ============
            with (
                tc.tile_pool(name="pa", bufs=1) as pa,
                tc.tile_pool(name="pa_wk", bufs=3) as pwk,
            ):
                ctx_f = pa.tile([128, 8, SK], bf16, tag="ctx")
                nc.sync.dma_start(ctx_f[:], r8(d_ctxT))
                bv_row = pa.tile([1, D], f32r, tag="bv_row")
                nc.sync.dma_start(bv_row[:], vec2d("bv").bitcast(f32r))
                bmat_row = pa.tile([1, D], f32r, tag="bmat_row")
                nc.sync.dma_start(bmat_row[:], vec2d("bmat").bitcast(f32r))
                nc.gpsimd.memset(v_aug[:, :, :, 0:64], 0.0)
                nc.gpsimd.memset(v_aug[:, :, :, 0:1], 1.0)
                wv_f = pa.tile([128, 8, D], bf16, tag="wv")
                wm_f = pa.tile([128, 8, D], bf16, tag="wm")

                # kT[ot] = WkT.T @ ctxT + bk, two 512-col halves
                for ot in range(8):
                    wk_c = pwk.tile([128, 8, 128], bf16, tag="wk")
                    nc.sync.dma_start(
                        wk_c[:], r8(d_w["WkT"])[:, :, ot * 128:(ot + 1) * 128]
                    )
                    for sh in range(2):
                        shs = slice(sh * 512, (sh + 1) * 512)
                        ps = pap.tile([128, 512], f32, tag="mm")
                        for i in range(8):
                            nc.tensor.matmul(
                                ps[:], wk_c[:, i, :], ctx_f[:, i, shs],
                                start=(i == 0), stop=(i == 7),
                            )
                        nc.vector.tensor_scalar_add(
                            kT[:, ot, shs], ps[:], bk_sc[:, ot:ot + 1]
                        )
                    if ot == 5:
                        nc.sync.dma_start(wv_f[:], r8(d_w["WvT"]))
                    if ot == 7:
                        nc.sync.dma_start(wm_f[:], r8(d_w["WmatT"]))

                # v rows (bias via rank-1 ones x bv); ones col 0 already set
                for oc in range(2):
                    for tt in range(8):
                        ps = pap.tile([128, 512], f32, tag="mm")
                        for i in range(8):
                            nc.tensor.matmul(
                                ps[:], ctx_f[:, i, tt * 128:(tt + 1) * 128],
                                wv_f[:, i, oc * 512:(oc + 1) * 512],
                                start=(i == 0), stop=False,
                            )
                        nc.tensor.matmul(
                            ps[:], ones_row[:], bv_row[:, oc * 512:(oc + 1) * 512],
                            start=False, stop=True,
                        )
                        nc.vector.tensor_copy(
                            v_aug[:, tt, oc * 8:(oc + 1) * 8, 64:128],
                            ps[:].rearrange("p (h j) -> p h j", j=DH),
                        )

                # new_k: relu(k @ Wmat.T + bmat) -> LN -> transpose -> nkT
                for grp in range(2):
                    tmpk = ptmp.tile([128, 4, 1024], f16, tag="tmp")
                    stats = ptmp.tile([128, 4, 2, 6], f32, tag="stats")
                    mv = ptmp.tile([128, 4, 2], f32, tag="mv")
                    for st in range(4):
                        tt = grp * 4 + st
                        for oc in range(2):
                            ps = pap.tile([128, 512], f32, tag="mm")
                            for i in range(8):
                                nc.tensor.matmul(
                                    ps[:], kT[:, i, tt * 128:(tt + 1) * 128],
                                    wm_f[:, i, oc * 512:(oc + 1) * 512],
                                    start=(i == 0), stop=False,
                                )
                            nc.tensor.matmul(
                                ps[:], ones_row[:],
                                bmat_row[:, oc * 512:(oc + 1) * 512],
                                start=False, stop=True,
                            )
                            nc.scalar.activation(
                                tmpk[:, st, oc * 512:(oc + 1) * 512], ps[:], AF.Relu
                            )
                            nc.vector.bn_stats(
                                stats[:, st, oc, :],
                                tmpk[:, st, oc * 512:(oc + 1) * 512],
                            )
                        nc.vector.bn_aggr(mv[:, st, :], stats[:, st, :, :])
                    ln_group(tmpk, mv, eps_k[:, grp * 4:(grp + 1) * 4],
                             nkT, grp * 512)

            # ============ phase B: q path -> nqT (two s-halves) ============
            pwob_cm = tc.tile_pool(name="p_wobs", bufs=1)
            pwob = pwob_cm.__enter__()
            wobs_f = pwob.tile([128, 8, D], bf16, tag="wobs")
            nc.sync.dma_start(wobs_f[:], r8(d_w["WobsT"]))
            pbpr_cm = tc.tile_pool(name="pb_probs", bufs=1)
            pbpr = pbpr_cm.__enter__()

            def emit_B(sc):  # generator: yields at stage boundaries
                scs = slice(sc * 512, (sc + 1) * 512)
                hs_c = phs.tile([128, 8, 512], bf16, tag="hs", bufs=1)
                nc.sync.dma_start(hs_c[:], r8(d_hsT)[:, :, scs])
                qT_c = phs.tile([128, 8, 512], bf16, tag="qs")
                for ot in range(8):
                    wq_c = pwq.tile([128, 8, 128], bf16, tag="wq")
                    nc.sync.dma_start(
                        wq_c[:], r8(d_w["WqT"])[:, :, ot * 128:(ot + 1) * 128]
                    )
                    ps = pap.tile([128, 512], f32, tag="mm")
                    for i in range(8):
                        nc.tensor.matmul(
                            ps[:], wq_c[:, i, :], hs_c[:, i, :],
                            start=(i == 0), stop=(i == 7),
                        )
                    nc.vector.tensor_scalar_add(
                        qT_c[:, ot, :], ps[:], bq_sc[:, ot:ot + 1]
                    )
                yield
                # probsT = exp(scoresT / 32), bf16 (unnormalized)
                probs = pbpr.tile([128, 8, 512], bf16, tag="probs")
                for tt in range(8):
                    ps = pap.tile([128, 512], f32, tag="mm")
                    for oc in range(8):
                        nc.tensor.matmul(
                            ps[:], kT[:, oc, tt * 128:(tt + 1) * 128],
                            qT_c[:, oc, :],
                            start=(oc == 0), stop=(oc == 7),
                        )
                    nc.scalar.activation(
                        probs[:, tt, :], ps[:], AF.Exp, scale=1.0 / 32.0
                    )
                yield
                # column sums S[s]; spread to [s_p, s_c] via DRAM bounce
                psS = pap.tile([128, 512], f32, tag="mm")
                for tt in range(8):
                    nc.tensor.matmul(
                        psS[0:1, :], ones_col_bf[:], probs[:, tt, :],
                        start=(tt == 0), stop=(tt == 7),
                    )
                nc.vector.tensor_copy(S_row[:, scs], psS[0:1, :])
                nc.sync.dma_start(d_rs[scs], S_row[:, scs].bitcast(f32))
                nc.sync.dma_start(
                    Sc[:, sc * 4:(sc + 1) * 4],
                    d_rs[scs].rearrange("(c p) -> p c", p=128),
                )
                # gctxT[o, s] = v.T @ probsT (unnormalized); row-tiled pairs
                gctx = phs.tile([128, 8, 512], bf16, tag="qs")
                for ot in range(8):
                    ps = pap.tile([128, 512], f32, tag="mm")
                    for tt in range(8):
                        for hl in range(2):
                            nc.tensor.matmul(
                                ps[hl * 64:(hl + 1) * 64, :],
                                v_aug[:, tt, 2 * ot + hl, 64:128],
                                probs[:, tt, :],
                                start=(tt == 0), stop=(tt == 7),
                            )
                    nc.vector.tensor_copy(gctx[:, ot, :], ps[:])
                yield
                # preq = relu(gctx_raw @ WobsT + S*bobs); LN w/ eps*S^2
                tmpq = ptmp.tile([128, 4, 1024], f16, tag="tmp")
                statq = ptmp.tile([128, 4, 2, 6], f32, tag="stats")
                mvq = ptmp.tile([128, 4, 2], f32, tag="mv")
                for st in range(4):
                    if st == 2:
                        yield
                    gst = sc * 4 + st
                    for oc in range(2):
                        ps = pap.tile([128, 512], f32, tag="mm")
                        for i in range(8):
                            nc.tensor.matmul(
                                ps[:], gctx[:, i, st * 128:(st + 1) * 128],
                                wobs_f[:, i, oc * 512:(oc + 1) * 512],
                                start=(i == 0), stop=False,
                            )
                        nc.tensor.matmul(
                            ps[:], S_row[:, gst * 128:(gst + 1) * 128],
                            bobs_row[:, oc * 512:(oc + 1) * 512],
                            start=False, stop=True,
                        )
                        nc.scalar.activation(
                            tmpq[:, st, oc * 512:(oc + 1) * 512], ps[:], AF.Relu
                        )
                        nc.vector.bn_stats(
                            statq[:, st, oc, :],
                            tmpq[:, st, oc * 512:(oc + 1) * 512],
                        )
                    nc.vector.bn_aggr(mvq[:, st, :], statq[:, st, :, :])
                yield
                # eps' = eps * S^2 for this half
                ecols = Sc[:, sc * 4:(sc + 1) * 4]
                eq = eps_q[:, sc * 4:(sc + 1) * 4]
                nc.vector.tensor_mul(eq, ecols, ecols)
                nc.vector.tensor_scalar_mul(eq, eq, LN_EPS)
                ln_group(tmpq, mvq, eq, nqT, sc * 512)

            for _ in emit_B(0):
                pass

            # ============ phase D: 16-head MHA over query-halves ============
            def emit_D_half(sh, pdp, pds, psc_pool, po_pool):  # yields per pair
                shs = slice(sh * 512, (sh + 1) * 512)
                for hc in range(H // 2):
                    probs = pdp.tile([128, 8, 2, 512], bf16, tag="probs_h")
                    for tt in range(8):
                        ps = psc_pool.tile([128, 2, 512], f32, tag="psc")
                        nc.tensor.matmul(
                            ps[:, 0, :],
                            nkT[0:64, hc, tt * 128:(tt + 1) * 128],
                            nqT[0:64, hc, shs], start=True, stop=True,
                        )
                        nc.tensor.matmul(
                            ps[:, 1, :],
                            nkT[64:128, hc, tt * 128:(tt + 1) * 128],
                            nqT[64:128, hc, shs], start=True, stop=True,
                        )
                        nc.scalar.activation(
                            probs[:, tt, :, :], ps[:], AF.Exp, scale=1.0 / 8.0
                        )
                    for hl in range(2):
                        h = 2 * hc + hl
                        ps = po_pool.tile([128, 512], f32, tag="po")
                        for tt in range(8):
                            nc.tensor.matmul(
                                ps[:], v_aug[:, tt, h, :],
                                probs[:, tt, hl, :],
                                start=(tt == 0), stop=(tt == 7),
                            )
                        # sums on partition 0; 1/S straight off PSUM
                        rinv = pds.tile([1, 512], f32, tag="rinv")
                        nc.vector.reciprocal_approx_fast(rinv[:], ps[0:1, :])
                        nc.sync.dma_start(d_rinv[h:h + 1, shs], rinv[:])
                        rbc = pds.tile([128, 512], f32, tag="rbc")
                        nc.sync.dma_start(
                            rbc[64:128, :],
                            d_rinv[h:h + 1, shs].to_broadcast([64, 512]),
                        )
                        outF = pds.tile([128, 512], f32, tag="outF")
                        nc.vector.tensor_mul(
                            outF[64:128, :], ps[64:128, :], rbc[64:128, :]
                        )
                        nc.sync.dma_start(
                            d_outT[h * DH:(h + 1) * DH, shs], outF[64:128, :]
                        )
                    yield

            # interleave B1 stage emission with D0 pairs so the scheduler
            # keeps ACT fed with D0 exps while B1 owns most of the PE
            with (
                tc.tile_pool(name="pd_probs0", bufs=2) as pdp0,
                tc.tile_pool(name="pd_st0", bufs=2) as pds0,
                tc.tile_pool(name="ps_sc0", bufs=2, space="PSUM") as psc0,
                tc.tile_pool(name="ps_po0", bufs=2, space="PSUM") as po0,
            ):
                b1 = emit_B(1)
                d0 = emit_D_half(0, pdp0, pds0, psc0, po0)
                # weave: one D0 pair ahead of each B1 stage so ACT always
                # has an exp stream while B1 drives the PE
                for _ in range(7):  # 6 B1 stages + final drive
                    next(d0, None)
                    next(b1, None)
                for _ in d0:
                    pass

            # close phase-B pools before D1 so D1 gets wide PSUM pools
            pbpr_cm.__exit__(None, None, None)
            pwob_cm.__exit__(None, None, None)
            pmm_cm.__exit__(None, None, None)
            ptmp_cm.__exit__(None, None, None)
            pwq_cm.__exit__(None, None, None)
            phs_cm.__exit__(None, None, None)
            pab_cm.__exit__(None, None, None)

            with (
                tc.tile_pool(name="pd_probs1", bufs=3) as pdp1,
                tc.tile_pool(name="pd_st1", bufs=4) as pds1,
                tc.tile_pool(name="ps_sc1", bufs=2, space="PSUM") as psc1,
                tc.tile_pool(name="ps_po1", bufs=4, space="PSUM") as po1,
            ):
                for _ in emit_D_half(1, pdp1, pds1, psc1, po1):
                    pass

    nc.compile()
    return nc


def _prep_in_maps(inputs):
    import ml_dtypes

    bf = ml_dtypes.bfloat16
    w = {
        "WqT": np.ascontiguousarray(np.asarray(inputs["Wq"]).T).astype(bf),
        "WkT": np.ascontiguousarray(np.asarray(inputs["Wk"]).T).astype(bf),
        "WvT": np.ascontiguousarray(np.asarray(inputs["Wv"]).T).astype(bf),
        "WobsT": np.ascontiguousarray(np.asarray(inputs["Wobs"]).T).astype(bf),
        "WmatT": np.ascontiguousarray(np.asarray(inputs["Wmat"]).T).astype(bf),
    }
    vecs = {
        k: np.ascontiguousarray(np.asarray(inputs[k], dtype=np.float32))
        for k in ("bq", "bk", "bv", "bobs", "bmat", "ln_g", "ln_b")
    }
    hs = np.asarray(inputs["hidden_states"])
    ctx = np.asarray(inputs["context"])
    in_maps = []
    for b in range(N_CORES):
        m = {
            "hsT": np.ascontiguousarray(hs[b].T).astype(bf),
            "ctxT": np.ascontiguousarray(ctx[b].T).astype(bf),
        }
        m.update(w)
        m.update(vecs)
        in_maps.append(m)
    return in_maps


def kernel(hidden_states, context, Wq, bq, Wk, bk, Wv, bv,
           Wobs, bobs, Wmat, bmat, ln_g, ln_b):
    from concourse import bass_utils

    if "nc" not in _CACHE:
        _CACHE["nc"] = _build()
    nc = _CACHE["nc"]

    in_maps = _prep_in_maps(dict(
        hidden_states=hidden_states, context=context, Wq=Wq, Wk=Wk, Wv=Wv,
        Wobs=Wobs, Wmat=Wmat, bq=bq, bk=bk, bv=bv, bobs=bobs, bmat=bmat,
        ln_g=ln_g, ln_b=ln_b,
    ))
    res = bass_utils.run_bass_kernel_spmd(nc, in_maps, core_ids=list(range(N_CORES)))
    out = np.stack([res.results[b]["outT"].T for b in range(N_CORES)], axis=0)
    return out.astype(np.float32)


# revision 20
# speedup vs baseline: 1.1574x; 1.0119x over previous
"""GuidedAttention Trainium2 kernel — batch-parallel over 8 NeuronCores.

Per core (one batch element, SQ=SK=D=1024, H=16, DH=64):
  q = hs @ Wq.T + bq ; k = ctx @ Wk.T + bk ; v = ctx @ Wv.T + bv
  attn1 = softmax(q k^T / 32) ; gctx = attn1 @ v
  new_q = LN(relu(gctx @ Wobs.T + bobs)) ; new_k = LN(relu(k @ Wmat.T + bmat))
  out = MHA(new_q, new_k, v)  (16 heads of 64)

v3 design (over the v2 all-SBUF bf16 kernel):
  * LN block transposes are ordinary matmuls against a f16 identity
    (stationary = the normalized fp16 block, FWL-eligible) instead of
    is_transpose PE ops — ~2.5x cheaper on PE; affine applied on 4
    batched blocks per PSUM bank.
  * ctx resident in SBUF; Wk streamed exactly once (v2 re-streamed it
    4x); kT emitted as two N=512 column halves.
  * MHA runs as two query-halves D0/D1.  D0 only depends on the first
    LN half, so the tile scheduler overlaps D0's exp stream (the ACT
    bottleneck) with phase B1's PE-bound work.
  * v_aug carries its softmax-sum ones column FIRST, so the per-head
    denominators land on PSUM partition 0 where reciprocal_approx_fast
    can read them directly — no DRAM bounces in the MHA tail.
Output is produced transposed [D, SQ]; the host transposes it back.
"""

import numpy as np

B, SQ, SK, D, H = 8, 1024, 1024, 1024, 16
DH = D // H
LN_EPS = 1e-5
N_CORES = 8

_CACHE = {}


def _build():
    import concourse.mybir as mybir
    import concourse.tile as tile
    from concourse import bacc
    from concourse.masks import make_identity

    f32 = mybir.dt.float32
    f32r = mybir.dt.float32r
    bf16 = mybir.dt.bfloat16
    f16 = mybir.dt.float16
    AF = mybir.ActivationFunctionType
    ALU = mybir.AluOpType

    nc = bacc.Bacc(None, target_bir_lowering=False)

    d_hsT = nc.dram_tensor("hsT", [D, SQ], bf16, kind="ExternalInput")
    d_ctxT = nc.dram_tensor("ctxT", [D, SK], bf16, kind="ExternalInput")
    d_w = {
        n: nc.dram_tensor(n, [D, D], bf16, kind="ExternalInput")
        for n in ("WqT", "WkT", "WvT", "WobsT", "WmatT")
    }
    d_b = {
        n: nc.dram_tensor(n, [D], f32, kind="ExternalInput")
        for n in ("bq", "bk", "bv", "bobs", "bmat", "ln_g", "ln_b")
    }
    d_outT = nc.dram_tensor("outT", [D, SQ], f32, kind="ExternalOutput")
    d_rs = nc.dram_tensor("rs_scratch", [SQ], f32, kind="Internal")
    d_rinv = nc.dram_tensor("rinv_scratch", [H, SQ], f32, kind="Internal")

    def r8(ap):  # [(c p), x] -> [p, c, x]
        return ap.rearrange("(c p) x -> p c x", p=128)

    def vec2d(name):  # [D] -> [1, D] AP
        return d_b[name][:].rearrange("(a d) -> a d", a=1)

    with tile.TileContext(nc) as tc:
        with tc.tile_pool(name="persist", bufs=1) as pp:
            # ---- constants / persistent tiles ----
            v_aug = pp.tile([128, 8, H, 128], bf16, tag="vaug")
            nqT = pp.tile([128, 8, SQ], bf16, tag="nqT")
            nkT = pp.tile([128, 8, SK], bf16, tag="nkT")
            ident = pp.tile([128, 128], f32, tag="ident")
            make_identity(nc, ident[:])
            ident_h = pp.tile([128, 128], f16, tag="ident_h")
            nc.vector.tensor_copy(ident_h[:], ident[:])
            ones_row = pp.tile([1, 128], f32r, tag="ones_row")
            d_ones = nc.inline_tensor(np.ones((1, 128), np.float32), name="ones_const")
            nc.sync.dma_start(ones_row[:], d_ones[:].bitcast(f32r))
            ones_col_bf = pp.tile([128, 1], bf16, tag="ones_col")
            nc.vector.memset(ones_col_bf[:], 1.0)
            bq_sc = pp.tile([128, 8], f32, tag="bq_sc")
            bk_sc = pp.tile([128, 8], f32, tag="bk_sc")
            nc.sync.dma_start(bq_sc[:], d_b["bq"][:].rearrange("(c p) -> p c", p=128))
            nc.sync.dma_start(bk_sc[:], d_b["bk"][:].rearrange("(c p) -> p c", p=128))
            g_sc = pp.tile([128, 8], f32, tag="g_sc")
            b_sc = pp.tile([128, 8], f32, tag="b_sc")
            nc.sync.dma_start(g_sc[:], d_b["ln_g"][:].rearrange("(c p) -> p c", p=128))
            nc.sync.dma_start(b_sc[:], d_b["ln_b"][:].rearrange("(c p) -> p c", p=128))
            bobs_row = pp.tile([1, D], f32r, tag="bobs_row")
            nc.sync.dma_start(bobs_row[:], vec2d("bobs").bitcast(f32r))
            # guide softmax column sums, [s_p, s_c] layout + derived eps'
            Sc = pp.tile([128, 8], f32, tag="Sc")
            eps_q = pp.tile([128, 8], f32, tag="eps_q")
            S_row = pp.tile([1, SQ], f32r, tag="S_row")

            def ln_batch_apply(tmp, n_str, var_all, mean_all, eps_col, tpool,
                               mm_pool, dst, dst_off):
                """tmp [128, n_str, 1024] fp16 relu'd rows (s on partitions),
                per-stripe stats APs [128, n_str].  Normalize in place,
                transpose 128x128 blocks via identity-matmul, then
                per-partition affine (g, b) into dst[:, ot, dst_off...]."""
                lnv = tpool.tile([128, n_str], f32, tag="lnv")
                nc.vector.tensor_tensor(lnv[:], var_all, eps_col, ALU.add)
                std = tpool.tile([128, n_str], f32, tag="std")
                nc.scalar.activation(std[:], lnv[:], AF.Sqrt)
                rstd = tpool.tile([128, n_str], f32, tag="rstd")
                nc.vector.reciprocal(rstd[:], std[:])
                negmr = tpool.tile([128, n_str], f32, tag="negmr")
                nc.vector.tensor_mul(negmr[:], mean_all, rstd[:])
                nc.vector.tensor_scalar_mul(negmr[:], negmr[:], -1.0)
                for i in range(n_str):
                    nc.vector.tensor_scalar(
                        tmp[:, i, :], tmp[:, i, :],
                        rstd[:, i:i + 1], negmr[:, i:i + 1],
                        ALU.mult, ALU.add,
                    )
                # transpose: out[d, s] = sum_s' tmp[s', d] * I[s', s]
                for ot in range(8):
                    pst = mm_pool.tile([128, 512], f32, tag="mm")
                    for i in range(n_str):
                        nc.tensor.matmul(
                            pst[:, i * 128:(i + 1) * 128],
                            tmp[:, i, ot * 128:(ot + 1) * 128],
                            ident_h[:], start=True, stop=True,
                        )
                    nc.vector.tensor_scalar(
                        dst[:, ot, dst_off:dst_off + n_str * 128],
                        pst[:], g_sc[:, ot:ot + 1], b_sc[:, ot:ot + 1],
                        ALU.mult, ALU.add,
                    )

            # ============ shared pools across A+B ============
            pab_cm = tc.tile_pool(name="pab", bufs=1)
            pab = pab_cm.__enter__()
            kT = pab.tile([128, 8, SK], bf16, tag="kT")
            eps_k = pab.tile([128, 8], f32, tag="eps_k")
            nc.vector.memset(eps_k[:], LN_EPS)
            phs_cm = tc.tile_pool(name="p_hs", bufs=2)
            phs = phs_cm.__enter__()
            pwq_cm = tc.tile_pool(name="p_wq", bufs=3)
            pwq = pwq_cm.__enter__()
            ptmp_cm = tc.tile_pool(name="p_tmp", bufs=2)
            ptmp = ptmp_cm.__enter__()
            pmm_cm = tc.tile_pool(name="ps_mm", bufs=2, space="PSUM")
            pap = pmm_cm.__enter__()

            def ln_group(tmp, mv, eps_col, dst, dst_off):
                ln_batch_apply(tmp, 4, mv[:, :, 1], mv[:, :, 0], eps_col,
                               ptmp, pap, dst, dst_off)

            # ============ phase A: kT, v, new_kT ============
            with (
                tc.tile_pool(name="pa", bufs=1) as pa,
                tc.tile_pool(name="pa_wk", bufs=3) as pwk,
            ):
                ctx_f = pa.tile([128, 8, SK], bf16, tag="ctx")
                for ic in range(4):
                    nc.sync.dma_start(
                        ctx_f[:, 2 * ic:2 * ic + 2, :],
                        r8(d_ctxT)[:, 2 * ic:2 * ic + 2, :],
                    )
                bv_row = pa.tile([1, D], f32r, tag="bv_row")
                nc.sync.dma_start(bv_row[:], vec2d("bv").bitcast(f32r))
                bmat_row = pa.tile([1, D], f32r, tag="bmat_row")
                nc.sync.dma_start(bmat_row[:], vec2d("bmat").bitcast(f32r))
                nc.gpsimd.memset(v_aug[:, :, :, 0:64], 0.0)
                nc.gpsimd.memset(v_aug[:, :, :, 0:1], 1.0)
                wv_f = pa.tile([128, 8, D], bf16, tag="wv")
                wm_f = pa.tile([128, 8, D], bf16, tag="wm")

                # kT[ot] = WkT.T @ ctxT + bk, two 512-col halves
                for ot in range(8):
                    wk_c = pwk.tile([128, 8, 128], bf16, tag="wk")
                    nc.sync.dma_start(
                        wk_c[:], r8(d_w["WkT"])[:, :, ot * 128:(ot + 1) * 128]
                    )
                    for sh in range(2):
                        shs = slice(sh * 512, (sh + 1) * 512)
                        ps = pap.tile([128, 512], f32, tag="mm")
                        for i in range(8):
                            nc.tensor.matmul(
                                ps[:], wk_c[:, i, :], ctx_f[:, i, shs],
                                start=(i == 0), stop=(i == 7),
                            )
                        nc.scalar.activation(
                            kT[:, ot, shs], ps[:], AF.Identity,
                            bias=bk_sc[:, ot:ot + 1],
                        )
                    if ot in (4, 5):
                        h4 = slice((ot - 4) * 4, (ot - 3) * 4)
                        nc.sync.dma_start(wv_f[:, h4, :], r8(d_w["WvT"])[:, h4, :])
                    if ot in (6, 7):
                        h4 = slice((ot - 6) * 4, (ot - 5) * 4)
                        nc.sync.dma_start(wm_f[:, h4, :], r8(d_w["WmatT"])[:, h4, :])

                # v rows (bias via rank-1 ones x bv); ones col 0 already set
                for oc in range(2):
                    for tt in range(8):
                        ps = pap.tile([128, 512], f32, tag="mm")
                        for i in range(8):
                            nc.tensor.matmul(
                                ps[:], ctx_f[:, i, tt * 128:(tt + 1) * 128],
                                wv_f[:, i, oc * 512:(oc + 1) * 512],
                                start=(i == 0), stop=False,
                            )
                        nc.tensor.matmul(
                            ps[:], ones_row[:], bv_row[:, oc * 512:(oc + 1) * 512],
                            start=False, stop=True,
                        )
                        nc.scalar.activation(
                            v_aug[:, tt, oc * 8:(oc + 1) * 8, 64:128],
                            ps[:].rearrange("p (h j) -> p h j", j=DH),
                            AF.Copy,
                        )

                # new_k: relu(k @ Wmat.T + bmat) -> LN -> transpose -> nkT
                for grp in range(2):
                    tmpk = ptmp.tile([128, 4, 1024], f16, tag="tmp")
                    stats = ptmp.tile([128, 4, 2, 6], f32, tag="stats")
                    mv = ptmp.tile([128, 4, 2], f32, tag="mv")
                    for st in range(4):
                        tt = grp * 4 + st
                        for oc in range(2):
                            ps = pap.tile([128, 512], f32, tag="mm")
                            for i in range(8):
                                nc.tensor.matmul(
                                    ps[:], kT[:, i, tt * 128:(tt + 1) * 128],
                                    wm_f[:, i, oc * 512:(oc + 1) * 512],
                                    start=(i == 0), stop=False,
                                )
                            nc.tensor.matmul(
                                ps[:], ones_row[:],
                                bmat_row[:, oc * 512:(oc + 1) * 512],
                                start=False, stop=True,
                            )
                            nc.scalar.activation(
                                tmpk[:, st, oc * 512:(oc + 1) * 512], ps[:], AF.Relu
                            )
                            nc.vector.bn_stats(
                                stats[:, st, oc, :],
                                tmpk[:, st, oc * 512:(oc + 1) * 512],
                            )
                        nc.vector.bn_aggr(mv[:, st, :], stats[:, st, :, :])
                    ln_group(tmpk, mv, eps_k[:, grp * 4:(grp + 1) * 4],
                             nkT, grp * 512)

            # ============ phase B: q path -> nqT (two s-halves) ============
            pwob_cm = tc.tile_pool(name="p_wobs", bufs=1)
            pwob = pwob_cm.__enter__()
            wobs_f = pwob.tile([128, 8, D], bf16, tag="wobs")
            for ic in range(2):
                h4 = slice(ic * 4, ic * 4 + 4)
                nc.sync.dma_start(wobs_f[:, h4, :], r8(d_w["WobsT"])[:, h4, :])
            pbpr_cm = tc.tile_pool(name="pb_probs", bufs=1)
            pbpr = pbpr_cm.__enter__()

            def emit_B(sc):  # generator: yields at stage boundaries
                scs = slice(sc * 512, (sc + 1) * 512)
                hs_c = phs.tile([128, 8, 512], bf16, tag="hs", bufs=1)
                for ic in range(2):
                    h4 = slice(ic * 4, ic * 4 + 4)
                    nc.sync.dma_start(hs_c[:, h4, :], r8(d_hsT)[:, h4, scs])
                qT_c = phs.tile([128, 8, 512], bf16, tag="qs")
                for ot in range(8):
                    wq_c = pwq.tile([128, 8, 128], bf16, tag="wq")
                    nc.sync.dma_start(
                        wq_c[:], r8(d_w["WqT"])[:, :, ot * 128:(ot + 1) * 128]
                    )
                    ps = pap.tile([128, 512], f32, tag="mm")
                    for i in range(8):
                        nc.tensor.matmul(
                            ps[:], wq_c[:, i, :], hs_c[:, i, :],
                            start=(i == 0), stop=(i == 7),
                        )
                    nc.scalar.activation(
                        qT_c[:, ot, :], ps[:], AF.Identity,
                        bias=bq_sc[:, ot:ot + 1],
                    )
                yield
                # probsT = exp(scoresT / 32), bf16 (unnormalized)
                probs = pbpr.tile([128, 8, 512], bf16, tag="probs")
                for tt in range(8):
                    ps = pap.tile([128, 512], f32, tag="mm")
                    for oc in range(8):
                        nc.tensor.matmul(
                            ps[:], kT[:, oc, tt * 128:(tt + 1) * 128],
                            qT_c[:, oc, :],
                            start=(oc == 0), stop=(oc == 7),
                        )
                    nc.scalar.activation(
                        probs[:, tt, :], ps[:], AF.Exp, scale=1.0 / 32.0
                    )
                yield
                # column sums S[s]; spread to [s_p, s_c] via DRAM bounce
                psS = pap.tile([128, 512], f32, tag="mm")
                for tt in range(8):
                    nc.tensor.matmul(
                        psS[0:1, :], ones_col_bf[:], probs[:, tt, :],
                        start=(tt == 0), stop=(tt == 7),
                    )
                nc.vector.tensor_copy(S_row[:, scs], psS[0:1, :])
                nc.sync.dma_start(d_rs[scs], S_row[:, scs].bitcast(f32))
                nc.sync.dma_start(
                    Sc[:, sc * 4:(sc + 1) * 4],
                    d_rs[scs].rearrange("(c p) -> p c", p=128),
                )
                # gctxT[o, s] = v.T @ probsT (unnormalized); row-tiled pairs
                gctx = phs.tile([128, 8, 512], bf16, tag="qs")
                for ot in range(8):
                    ps = pap.tile([128, 512], f32, tag="mm")
                    for tt in range(8):
                        for hl in range(2):
                            nc.tensor.matmul(
                                ps[hl * 64:(hl + 1) * 64, :],
                                v_aug[:, tt, 2 * ot + hl, 64:128],
                                probs[:, tt, :],
                                start=(tt == 0), stop=(tt == 7),
                            )
                    nc.scalar.activation(gctx[:, ot, :], ps[:], AF.Copy)
                yield
                # preq = relu(gctx_raw @ WobsT + S*bobs); LN w/ eps*S^2
                tmpq = ptmp.tile([128, 4, 1024], f16, tag="tmp")
                statq = ptmp.tile([128, 4, 2, 6], f32, tag="stats")
                mvq = ptmp.tile([128, 4, 2], f32, tag="mv")
                for st in range(4):
                    if st == 2:
                        yield
                    gst = sc * 4 + st
                    for oc in range(2):
                        ps = pap.tile([128, 512], f32, tag="mm")
                        for i in range(8):
                            nc.tensor.matmul(
                                ps[:], gctx[:, i, st * 128:(st + 1) * 128],
                                wobs_f[:, i, oc * 512:(oc + 1) * 512],
                                start=(i == 0), stop=False,
                            )
                        nc.tensor.matmul(
                            ps[:], S_row[:, gst * 128:(gst + 1) * 128],
                            bobs_row[:, oc * 512:(oc + 1) * 512],
                            start=False, stop=True,
                        )
                        nc.scalar.activation(
                            tmpq[:, st, oc * 512:(oc + 1) * 512], ps[:], AF.Relu
                        )
                        nc.vector.bn_stats(
                            statq[:, st, oc, :],
                            tmpq[:, st, oc * 512:(oc + 1) * 512],
                        )
                    nc.vector.bn_aggr(mvq[:, st, :], statq[:, st, :, :])
                yield
                # eps' = eps * S^2 for this half
                ecols = Sc[:, sc * 4:(sc + 1) * 4]
                eq = eps_q[:, sc * 4:(sc + 1) * 4]
                nc.vector.tensor_mul(eq, ecols, ecols)
                nc.vector.tensor_scalar_mul(eq, eq, LN_EPS)
                ln_group(tmpq, mvq, eq, nqT, sc * 512)

            for _ in emit_B(0):
                pass

            # ============ phase D: 16-head MHA over query-halves ============
            def emit_D_half(sh, pdp, pds, psc_pool, po_pool):  # yields per pair
                shs = slice(sh * 512, (sh + 1) * 512)
                for hc in range(H // 2):
                    probs = pdp.tile([128, 8, 2, 512], bf16, tag="probs_h")
                    for tt in range(8):
                        ps = psc_pool.tile([128, 2, 512], f32, tag="psc")
                        nc.tensor.matmul(
                            ps[:, 0, :],
                            nkT[0:64, hc, tt * 128:(tt + 1) * 128],
                            nqT[0:64, hc, shs], start=True, stop=True,
                        )
                        nc.tensor.matmul(
                            ps[:, 1, :],
                            nkT[64:128, hc, tt * 128:(tt + 1) * 128],
                            nqT[64:128, hc, shs], start=True, stop=True,
                        )
                        nc.scalar.activation(
                            probs[:, tt, :, :], ps[:], AF.Exp, scale=1.0 / 8.0
                        )
                    for hl in range(2):
                        h = 2 * hc + hl
                        ps = po_pool.tile([128, 512], f32, tag="po")
                        for tt in range(8):
                            nc.tensor.matmul(
                                ps[:], v_aug[:, tt, h, :],
                                probs[:, tt, hl, :],
                                start=(tt == 0), stop=(tt == 7),
                            )
                        # sums on partition 0; 1/S straight off PSUM.
                        # outU copy releases the PSUM tile before the
                        # rinv DMA round-trip completes.
                        rinv = pds.tile([1, 512], f32, tag="rinv")
                        nc.vector.reciprocal_approx_fast(rinv[:], ps[0:1, :])
                        nc.sync.dma_start(d_rinv[h:h + 1, shs], rinv[:])
                        outU = pds.tile([128, 512], f32, tag="outU", bufs=3)
                        nc.vector.tensor_copy(outU[64:128, :], ps[64:128, :])
                        rbc = pds.tile([128, 512], f32, tag="rbc")
                        nc.sync.dma_start(
                            rbc[64:128, :],
                            d_rinv[h:h + 1, shs].to_broadcast([64, 512]),
                        )
                        nc.vector.tensor_mul(
                            outU[64:128, :], outU[64:128, :], rbc[64:128, :]
                        )
                        nc.sync.dma_start(
                            d_outT[h * DH:(h + 1) * DH, shs], outU[64:128, :]
                        )
                    yield

            # interleave B1 stage emission with D0 pairs so the scheduler
            # keeps ACT fed with D0 exps while B1 owns most of the PE
            with (
                tc.tile_pool(name="pd_probs0", bufs=2) as pdp0,
                tc.tile_pool(name="pd_st0", bufs=2) as pds0,
                tc.tile_pool(name="ps_sc0", bufs=2, space="PSUM") as psc0,
                tc.tile_pool(name="ps_po0", bufs=2, space="PSUM") as po0,
            ):
                b1 = emit_B(1)
                d0 = emit_D_half(0, pdp0, pds0, psc0, po0)
                # weave: one D0 pair ahead of each B1 stage so ACT always
                # has an exp stream while B1 drives the PE
                for _ in range(7):  # 6 B1 stages + final drive
                    next(d0, None)
                    next(b1, None)
                for _ in d0:
                    pass

            # close phase-B pools before D1 so D1 gets wide PSUM pools
            pbpr_cm.__exit__(None, None, None)
            pwob_cm.__exit__(None, None, None)
            pmm_cm.__exit__(None, None, None)
            ptmp_cm.__exit__(None, None, None)
            pwq_cm.__exit__(None, None, None)
            phs_cm.__exit__(None, None, None)
            pab_cm.__exit__(None, None, None)

            with (
                tc.tile_pool(name="pd_probs1", bufs=3) as pdp1,
                tc.tile_pool(name="pd_st1", bufs=4) as pds1,
                tc.tile_pool(name="ps_sc1", bufs=2, space="PSUM") as psc1,
                tc.tile_pool(name="ps_po1", bufs=4, space="PSUM") as po1,
            ):
                for _ in emit_D_half(1, pdp1, pds1, psc1, po1):
                    pass

    nc.compile()
    return nc


def _prep_in_maps(inputs):
    import ml_dtypes

    bf = ml_dtypes.bfloat16
    w = {
        "WqT": np.ascontiguousarray(np.asarray(inputs["Wq"]).T).astype(bf),
        "WkT": np.ascontiguousarray(np.asarray(inputs["Wk"]).T).astype(bf),
        "WvT": np.ascontiguousarray(np.asarray(inputs["Wv"]).T).astype(bf),
        "WobsT": np.ascontiguousarray(np.asarray(inputs["Wobs"]).T).astype(bf),
        "WmatT": np.ascontiguousarray(np.asarray(inputs["Wmat"]).T).astype(bf),
    }
    vecs = {
        k: np.ascontiguousarray(np.asarray(inputs[k], dtype=np.float32))
        for k in ("bq", "bk", "bv", "bobs", "bmat", "ln_g", "ln_b")
    }
    hs = np.asarray(inputs["hidden_states"])
    ctx = np.asarray(inputs["context"])
    in_maps = []
    for b in range(N_CORES):
        m = {
            "hsT": np.ascontiguousarray(hs[b].T).astype(bf),
            "ctxT": np.ascontiguousarray(ctx[b].T).astype(bf),
        }
        m.update(w)
        m.update(vecs)
        in_maps.append(m)
    return in_maps


def kernel(hidden_states, context, Wq, bq, Wk, bk, Wv, bv,
           Wobs, bobs, Wmat, bmat, ln_g, ln_b):
    from concourse import bass_utils

    if "nc" not in _CACHE:
        _CACHE["nc"] = _build()
    nc = _CACHE["nc"]

    in_maps = _prep_in_maps(dict(
        hidden_states=hidden_states, context=context, Wq=Wq, Wk=Wk, Wv=Wv,
        Wobs=Wobs, Wmat=Wmat, bq=bq, bk=bk, bv=bv, bobs=bobs, bmat=bmat,
        ln_g=ln_g, ln_b=ln_b,
    ))
    res = bass_utils.run_bass_kernel_spmd(nc, in_maps, core_ids=list(range(N_CORES)))
    out = np.stack([res.results[b]["outT"].T for b in range(N_CORES)], axis=0)
    return out.astype(np.float32)
